# revision 2
# baseline (speedup 1.0000x reference)
"""Multi-head attention (B=4, N=2048, C=1024, H=16) on 8 TRN2 NeuronCores. v2.

Sharding: (batch, head-group) pairs -> 8 cores. Core c handles batch c//2 and
heads [(c%2)*8, (c%2)*8+8). QKV weights column-sharded per head group, proj
row-sharded; each core emits a partial proj output (transposed); host sums the
two partials per batch and adds b_proj.

v2 changes vs baseline (each microbenchmarked on HW first):
  - scores: the two heads of a pair run CONCURRENTLY as PE row-tiles
    (tile_position (0,0)/(64,0), K=64), emitted interleaved T0,T64,T0,T64.
    Measured 172 ns/MM vs 271 serial.
  - exp split across engines per key-block: ACT exps one head's scores
    (exact LUT) while DVE exps the other head's via a Schraudolph bf16
    bit-trick (round(s*128/ln2 + 16248.5) as int16 = bf16 bits of e^s;
    1.79% rms measured). Parity alternates per block. Halves the former
    single-engine softmax bottleneck.
  - proj(j-1), qk-gen(j+1) and v-gen units interleave into the attention
    loop as PE fillers so the PE stays busy while exp streams.
  - proj PSUM->SBUF staging copies moved to ACT (Copy) to unload DVE.
"""

import os
import sys

import numpy as np

for _p in ("/root/.axon_site", "/root/.axon_site/_ro/trn_rl_repo",
           "/root/.axon_site/_ro/pypackages", "/opt/trn_rl_repo", "/opt/pypackages"):
    if os.path.isdir(_p) and _p not in sys.path:
        sys.path.append(_p)

import concourse.bacc as bacc
import concourse.mybir as mybir
import concourse.tile as tile
from concourse.bass_utils import run_bass_kernel_spmd

B, N, C = 4, 2048, 1024
H, D = 16, 64
NH = 8            # heads per core
CL = NH * D       # 512 local channels
NCORES = 8
SCALE = float(D) ** -0.5

F32 = mybir.dt.float32
BF16 = mybir.dt.bfloat16
I16 = mybir.dt.int16

MM_DT = "bf16"

# Schraudolph bf16 exp: bits16 = round(s * 128/ln2 + (127*128 - 7.5))
A_EXP = 184.66509399414062
B_EXP = 16248.5

_CACHE = {}


def build_nc():
    nc = bacc.Bacc()

    xT = nc.declare_dram_parameter("xT", [C, N], BF16, isOutput=False)
    wqk = nc.declare_dram_parameter("wqk", [C, 2 * CL], BF16, isOutput=False)
    wv = nc.declare_dram_parameter("wv", [C, CL], BF16, isOutput=False)
    wp = nc.declare_dram_parameter("wp", [CL, C], BF16, isOutput=False)
    bqk = nc.declare_dram_parameter("bqk", [128, 8], F32, isOutput=False)
    yT = nc.declare_dram_parameter("yT", [C, N], F32, isOutput=True)

    Ident = mybir.ActivationFunctionType.Identity
    Exp = mybir.ActivationFunctionType.Exp
    Copy = mybir.ActivationFunctionType.Copy
    Mult = mybir.AluOpType.mult
    Add = mybir.AluOpType.add

    with tile.TileContext(nc) as tc:
        with (
            tc.tile_pool(name="const", bufs=1) as const,
            tc.tile_pool(name="wpool", bufs=1) as wpool,
            tc.tile_pool(name="qkpool", bufs=1) as qkpool,
            tc.tile_pool(name="vpool", bufs=1) as vpool,
            tc.tile_pool(name="aopool", bufs=1) as aopool,
            tc.tile_pool(name="xpool", bufs=1) as xpool,
            tc.tile_pool(name="ptpool", bufs=16) as ptpool,
            tc.tile_pool(name="rpool", bufs=3) as rpool,
            tc.tile_pool(name="ytpool", bufs=4) as ytpool,
        ):
            bqk_t = const.tile([128, 8], F32, tag="bqk", name="bqk")
            nc.sync.dma_start(out=bqk_t[:], in_=bqk[:])

            wqk_t = []
            wv_t = []
            for cc in range(8):
                wt = wpool.tile([128, 2 * CL], BF16, tag=f"wqk{cc}", name=f"wqk{cc}")
                nc.sync.dma_start(out=wt[:], in_=wqk[cc * 128:(cc + 1) * 128, :])
                wqk_t.append(wt)
                vt = wpool.tile([128, CL], BF16, tag=f"wv{cc}", name=f"wv{cc}")
                nc.sync.dma_start(out=vt[:], in_=wv[cc * 128:(cc + 1) * 128, :])
                wv_t.append(vt)
            wp_t = []
            for cl in range(4):
                wt = wpool.tile([128, C], BF16, tag=f"wp{cl}", name=f"wp{cl}")
                nc.sync.dma_start(out=wt[:], in_=wp[cl * 128:(cl + 1) * 128, :])
                wp_t.append(wt)

            xT_t = []
            for cc in range(8):
                t = xpool.tile([128, N], BF16, tag=f"xT{cc}", name=f"xT{cc}")
                nc.sync.dma_start(out=t[:], in_=xT[cc * 128:(cc + 1) * 128, :])
                xT_t.append(t)

            # persistent intermediates
            qk_t = []   # m 0..3 -> q^T chunks (scaled+biased), 4..7 -> k^T
            for m in range(8):
                qk_t.append(qkpool.tile([128, N], BF16, tag=f"qk{m}", name=f"qk{m}"))
            v_t = []    # [v | 1 | pad] per head: 8 groups of 128 cols
            for kc in range(16):
                v_t.append(vpool.tile([128, NH * 128], BF16, tag=f"v{kc}", name=f"v{kc}"))
            ao_t = []   # ao_t[p]: pair p normalized out^T (128 ch x N)
            for p in range(4):
                ao_t.append(aopool.tile([128, N], BF16, tag=f"ao{p}", name=f"ao{p}"))

            # ---------------- unit builders ----------------
            def qk_unit(pool, m, j):
                js = slice(j * 512, (j + 1) * 512)
                ps = pool.tile([128, 512], F32, tag="ps", name="ps")
                for cc in range(8):
                    nc.tensor.matmul(
                        ps[:], wqk_t[cc][:, m * 128:(m + 1) * 128],
                        xT_t[cc][:, js], start=(cc == 0), stop=(cc == 7))
                nc.scalar.activation(
                    qk_t[m][:, js], ps[:], Ident,
                    bias=bqk_t[:, m:m + 1],
                    scale=SCALE if m < 4 else 1.0)

            def v_unit(pool, kc):
                ps = pool.tile([128, 512], F32, tag="ps", name="ps")
                for cc in range(8):
                    nc.tensor.matmul(
                        ps[:], xT_t[cc][:, kc * 128:(kc + 1) * 128],
                        wv_t[cc][:], start=(cc == 0), stop=(cc == 7))
                v3 = v_t[kc].rearrange("p (h e) -> p h e", h=NH)
                nc.vector.memset(v3[:, :, 64:128], 0.0)
                nc.vector.memset(v3[:, :, 64:65], 1.0)
                nc.vector.tensor_copy(
                    v3[:, :, 0:64],
                    ps.rearrange("p (h e) -> p h e", e=64))

            def proj_unit(pool, m2, j):
                js = slice(j * 512, (j + 1) * 512)
                py = pool.tile([128, 512], F32, tag="ps", name="py")
                for cl in range(4):
                    nc.tensor.matmul(
                        py[:], wp_t[cl][:, m2 * 128:(m2 + 1) * 128],
                        ao_t[cl][:, js], start=(cl == 0), stop=(cl == 3))
                yt = ytpool.tile([128, 512], F32, tag="yt", name="yt")
                nc.scalar.activation(yt[:], py[:], Copy)
                nc.sync.dma_start(out=yT[m2 * 128:(m2 + 1) * 128, js], in_=yt[:])

            # ---------------- prologue ----------------
            with tc.tile_pool(name="psAB", bufs=6, space="PSUM") as psAB:
                for m in range(4, 8):
                    for jj in range(4):
                        qk_unit(psAB, m, jj)
                for m in range(4):
                    qk_unit(psAB, m, 0)
                for kc in range(8):
                    v_unit(psAB, kc)

            # ---------------- attention + interleaved fillers ----------------
            # per key-chunk kc: head-even exp on ACT (exact), head-odd exp on
            # DVE (Schraudolph). Separate engines overlap, and each parity's
            # ss WAR chain resolves at a steady per-engine pace so the
            # row-pair score MMs stay synchronized.
            with (
                tc.tile_pool(name="ssE", bufs=2, space="PSUM") as ssEp,
                tc.tile_pool(name="ssO", bufs=2, space="PSUM") as ssOp,
                tc.tile_pool(name="avE", bufs=1, space="PSUM") as avEp,
                tc.tile_pool(name="avO", bufs=1, space="PSUM") as avOp,
                tc.tile_pool(name="aux", bufs=2, space="PSUM") as aux,
            ):
                fill_q = {
                    0: [("q", m, 1) for m in range(4)],
                    1: ([("proj", m2, 0) for m2 in range(8)]
                        + [("q", m, 2) for m in range(4)]),
                    2: ([("proj", m2, 1) for m2 in range(8)]
                        + [("q", m, 3) for m in range(4)]),
                    3: [("proj", m2, 2) for m2 in range(8)],
                }

                pend = [None]
                for j in range(4):
                    js = slice(j * 512, (j + 1) * 512)
                    queue = fill_q[j]
                    fstate = {"i": 0}

                    def filler():
                        if fstate["i"] < len(queue):
                            kind, a, b = queue[fstate["i"]]
                            fstate["i"] += 1
                            if kind == "proj":
                                proj_unit(aux, a, b)
                            elif kind == "v":
                                v_unit(aux, a)
                            else:  # "q" or "k"
                                qk_unit(aux, a if kind == "q" else a, b)

                    def sc_pair(p, kc):
                        ssE = ssEp.tile([128, 512], F32, tag="ssE", name="ssE")
                        ssO = ssOp.tile([128, 512], F32, tag="ssO", name="ssO")
                        ks = slice(kc * 128, (kc + 1) * 128)
                        nc.tensor.matmul(
                            ssE[:], qk_t[4 + p][0:64, ks],
                            qk_t[p][0:64, js], start=True, stop=True,
                            tile_position=(0, 0))
                        nc.tensor.matmul(
                            ssO[:], qk_t[4 + p][64:128, ks],
                            qk_t[p][64:128, js], start=True, stop=True,
                            tile_position=(64, 0))
                        return ssE, ssO

                    def exp_pair(ssE, ssO, kc):
                        ptE = ptpool.tile([128, 512], BF16, tag="ptE", name="ptE")
                        ptO = ptpool.tile([128, 512], BF16, tag="ptO", name="ptO")
                        nc.scalar.activation(ptE[:], ssE[:], Exp)
                        if kc < 15:
                            nc.vector.tensor_scalar(
                                ptO.bitcast(I16)[:], ssO[:], A_EXP, B_EXP, Mult, Add)
                        else:
                            # free the DVE for the cross-pair normalization
                            nc.scalar.activation(ptO[:], ssO[:], Exp)
                        return ptE, ptO

                    def av_pair(avE, avO, p, kc, pts):
                        ptE, ptO = pts[kc]
                        he, ho = 2 * p, 2 * p + 1
                        nc.tensor.matmul(
                            avE[:], v_t[kc][:, he * 128:(he + 1) * 128],
                            ptE[:], start=(kc == 0), stop=(kc == 15))
                        nc.tensor.matmul(
                            avO[:], v_t[kc][:, ho * 128:(ho + 1) * 128],
                            ptO[:], start=(kc == 0), stop=(kc == 15))

                    def norm_head(av, p, po, js_):
                        # av rows 0:64 = out^T, row 64 = Z; write ao rows po:po+64
                        z1 = rpool.tile([1, 512], F32, tag="z1", name="z1")
                        nc.vector.tensor_copy(z1[:], av[64:65, :])
                        r1 = rpool.tile([1, 512], F32, tag="r1", name="r1")
                        nc.vector.reciprocal_approx_fast(out=r1[:], in_=z1[:])
                        rb = rpool.tile([64, 512], F32, tag="rb", name="rb")
                        nc.gpsimd.partition_broadcast(rb[:], r1[:])
                        nc.vector.tensor_mul(
                            ao_t[p][po:po + 64, js_], av[0:64, :], rb[:])

                    for p in range(4):
                        pts = {}
                        avE = avO = None
                        for kc in range(16):
                            pts[kc] = exp_pair(*sc_pair(p, kc), kc)
                            if kc == 1 and pend[0] is not None:
                                pavE, pavO, pp, pjs = pend[0]
                                norm_head(pavE, pp, 0, pjs)
                                norm_head(pavO, pp, 64, pjs)
                                pend[0] = None
                            if kc == 2:
                                avE = avEp.tile([128, 512], F32, tag="avE",
                                                name="avE")
                                avO = avOp.tile([128, 512], F32, tag="avO",
                                                name="avO")
                            if kc >= 2:
                                av_pair(avE, avO, p, kc - 2, pts)
                            if j == 0 and p == 0:
                                if kc < 8:
                                    v_unit(aux, 8 + kc)
                            elif kc % 4 == 3:
                                filler()
                        av_pair(avE, avO, p, 14, pts)
                        av_pair(avE, avO, p, 15, pts)
                        pend[0] = (avE, avO, p, js)

                    while fstate["i"] < len(queue):
                        filler()

                    if j == 3:
                        pavE, pavO, pp, pjs = pend[0]
                        norm_head(pavE, pp, 0, pjs)
                        norm_head(pavO, pp, 64, pjs)
                        pend[0] = None

                # tail: proj for j=3
                for m2 in range(8):
                    proj_unit(aux, m2, 3)

    nc.compile()
    return nc


def make_in_maps(x, w_qkv, b_qkv, w_proj):
    np_bf = mybir.dt.np(BF16)
    x = np.asarray(x, np.float32)
    w_qkv = np.asarray(w_qkv, np.float32)
    b_qkv = np.asarray(b_qkv, np.float32)
    w_proj = np.asarray(w_proj, np.float32)
    in_maps = []
    for c in range(NCORES):
        b, g = divmod(c, 2)
        h0 = g * NH
        qs = slice(h0 * D, h0 * D + CL)
        ks = slice(C + h0 * D, C + h0 * D + CL)
        vs = slice(2 * C + h0 * D, 2 * C + h0 * D + CL)
        wqk_m = np.concatenate([w_qkv[:, qs], w_qkv[:, ks]], axis=1)
        bq = b_qkv[qs] * SCALE
        bk = b_qkv[ks]
        bqk_m = np.concatenate([bq, bk]).reshape(8, 128).T  # [128, 8] col-chunks
        in_maps.append({
            "xT": np.ascontiguousarray(x[b].T).astype(np_bf),
            "wqk": np.ascontiguousarray(wqk_m).astype(np_bf),
            "wv": np.ascontiguousarray(w_qkv[:, vs]).astype(np_bf),
            "wp": np.ascontiguousarray(w_proj[h0 * D:h0 * D + CL, :]).astype(np_bf),
            "bqk": np.ascontiguousarray(bqk_m, np.float32),
        })
    return in_maps


def run(x, w_qkv, b_qkv, w_proj, b_proj, mm_dt=MM_DT, **spmd_kwargs):
    if "nc" not in _CACHE:
        _CACHE["nc"] = build_nc()
    nc = _CACHE["nc"]
    in_maps = make_in_maps(x, w_qkv, b_qkv, w_proj)
    res = run_bass_kernel_spmd(nc, in_maps, core_ids=list(range(NCORES)),
                               **spmd_kwargs)
    # v-bias passes through softmax averaging exactly (weights sum to 1),
    # so its projected contribution folds into the output bias on the host.
    b_eff = (np.asarray(b_proj, np.float64)
             + np.asarray(b_qkv, np.float64)[2 * C:] @ np.asarray(w_proj, np.float64)
             ).astype(np.float32)
    out = np.empty((B, N, C), np.float32)
    for b in range(B):
        acc = res.results[2 * b]["yT"] + res.results[2 * b + 1]["yT"]
        out[b] = acc.T + b_eff[None, :]
    return out, res


def kernel(x, w_qkv, b_qkv, w_proj, b_proj):
    out, _ = run(x, w_qkv, b_qkv, w_proj, b_proj)
    return out


# revision 3
# speedup vs baseline: 1.0095x; 1.0095x over previous
"""Multi-head attention (B=4, N=2048, C=1024, H=16) on 8 TRN2 NeuronCores. v2.

Sharding: (batch, head-group) pairs -> 8 cores. Core c handles batch c//2 and
heads [(c%2)*8, (c%2)*8+8). QKV weights column-sharded per head group, proj
row-sharded; each core emits a partial proj output (transposed); host sums the
two partials per batch and adds b_proj.

v2 changes vs baseline (each microbenchmarked on HW first):
  - scores: the two heads of a pair run CONCURRENTLY as PE row-tiles
    (tile_position (0,0)/(64,0), K=64), emitted interleaved T0,T64,T0,T64.
    Measured 172 ns/MM vs 271 serial.
  - exp split across engines per key-block: ACT exps one head's scores
    (exact LUT) while DVE exps the other head's via a Schraudolph bf16
    bit-trick (round(s*128/ln2 + 16248.5) as int16 = bf16 bits of e^s;
    1.79% rms measured). Parity alternates per block. Halves the former
    single-engine softmax bottleneck.
  - proj(j-1), qk-gen(j+1) and v-gen units interleave into the attention
    loop as PE fillers so the PE stays busy while exp streams.
  - proj PSUM->SBUF staging copies moved to ACT (Copy) to unload DVE.
"""

import os
import sys

import numpy as np

for _p in ("/root/.axon_site", "/root/.axon_site/_ro/trn_rl_repo",
           "/root/.axon_site/_ro/pypackages", "/opt/trn_rl_repo", "/opt/pypackages"):
    if os.path.isdir(_p) and _p not in sys.path:
        sys.path.append(_p)

import concourse.bacc as bacc
import concourse.mybir as mybir
import concourse.tile as tile
from concourse.bass_utils import run_bass_kernel_spmd

B, N, C = 4, 2048, 1024
H, D = 16, 64
NH = 8            # heads per core
CL = NH * D       # 512 local channels
NCORES = 8
SCALE = float(D) ** -0.5

F32 = mybir.dt.float32
BF16 = mybir.dt.bfloat16
I16 = mybir.dt.int16

MM_DT = "bf16"

# Schraudolph bf16 exp: bits16 = round(s * 128/ln2 + (127*128 - 7.5))
A_EXP = 184.66509399414062
B_EXP = 16248.5

_CACHE = {}


def build_nc():
    nc = bacc.Bacc()

    xT = nc.declare_dram_parameter("xT", [C, N], BF16, isOutput=False)
    wqk = nc.declare_dram_parameter("wqk", [C, 2 * CL], BF16, isOutput=False)
    wv = nc.declare_dram_parameter("wv", [C, CL], BF16, isOutput=False)
    wp = nc.declare_dram_parameter("wp", [CL, C], BF16, isOutput=False)
    bqk = nc.declare_dram_parameter("bqk", [128, 8], F32, isOutput=False)
    yT = nc.declare_dram_parameter("yT", [C, N], F32, isOutput=True)

    Ident = mybir.ActivationFunctionType.Identity
    Exp = mybir.ActivationFunctionType.Exp
    Copy = mybir.ActivationFunctionType.Copy
    Mult = mybir.AluOpType.mult
    Add = mybir.AluOpType.add

    with tile.TileContext(nc) as tc:
        with (
            tc.tile_pool(name="const", bufs=1) as const,
            tc.tile_pool(name="wpool", bufs=1) as wpool,
            tc.tile_pool(name="qkpool", bufs=1) as qkpool,
            tc.tile_pool(name="vpool", bufs=1) as vpool,
            tc.tile_pool(name="aopool", bufs=1) as aopool,
            tc.tile_pool(name="xpool", bufs=1) as xpool,
            tc.tile_pool(name="ptpool", bufs=16) as ptpool,
            tc.tile_pool(name="rpool", bufs=3) as rpool,
            tc.tile_pool(name="ytpool", bufs=4) as ytpool,
        ):
            bqk_t = const.tile([128, 8], F32, tag="bqk", name="bqk")
            nc.sync.dma_start(out=bqk_t[:], in_=bqk[:])

            wqk_t = []
            wv_t = []
            for cc in range(8):
                wt = wpool.tile([128, 2 * CL], BF16, tag=f"wqk{cc}", name=f"wqk{cc}")
                nc.sync.dma_start(out=wt[:], in_=wqk[cc * 128:(cc + 1) * 128, :])
                wqk_t.append(wt)
                vt = wpool.tile([128, CL], BF16, tag=f"wv{cc}", name=f"wv{cc}")
                nc.sync.dma_start(out=vt[:], in_=wv[cc * 128:(cc + 1) * 128, :])
                wv_t.append(vt)
            wp_t = []
            for cl in range(4):
                wt = wpool.tile([128, C], BF16, tag=f"wp{cl}", name=f"wp{cl}")
                nc.sync.dma_start(out=wt[:], in_=wp[cl * 128:(cl + 1) * 128, :])
                wp_t.append(wt)

            xT_t = []
            for cc in range(8):
                t = xpool.tile([128, N], BF16, tag=f"xT{cc}", name=f"xT{cc}")
                nc.sync.dma_start(out=t[:], in_=xT[cc * 128:(cc + 1) * 128, :])
                xT_t.append(t)

            # persistent intermediates
            qk_t = []   # m 0..3 -> q^T chunks (scaled+biased), 4..7 -> k^T
            for m in range(8):
                qk_t.append(qkpool.tile([128, N], BF16, tag=f"qk{m}", name=f"qk{m}"))
            v_t = []    # [v | 1 | pad] per head: 8 groups of 128 cols
            for kc in range(16):
                v_t.append(vpool.tile([128, NH * 128], BF16, tag=f"v{kc}", name=f"v{kc}"))
            ao_t = []   # ao_t[p]: pair p normalized out^T (128 ch x N)
            for p in range(4):
                ao_t.append(aopool.tile([128, N], BF16, tag=f"ao{p}", name=f"ao{p}"))

            # ---------------- unit builders ----------------
            def qk_unit(pool, m, j, act_on_dve=False):
                js = slice(j * 512, (j + 1) * 512)
                ps = pool.tile([128, 512], F32, tag="ps", name="ps")
                for cc in range(8):
                    nc.tensor.matmul(
                        ps[:], wqk_t[cc][:, m * 128:(m + 1) * 128],
                        xT_t[cc][:, js], start=(cc == 0), stop=(cc == 7))
                if act_on_dve:
                    nc.vector.tensor_scalar(
                        qk_t[m][:, js], ps[:], SCALE if m < 4 else 1.0,
                        bqk_t[:, m:m + 1], Mult, Add)
                else:
                    nc.scalar.activation(
                        qk_t[m][:, js], ps[:], Ident,
                        bias=bqk_t[:, m:m + 1],
                        scale=SCALE if m < 4 else 1.0)

            def v_unit(pool, kc):
                ps = pool.tile([128, 512], F32, tag="ps", name="ps")
                for cc in range(8):
                    nc.tensor.matmul(
                        ps[:], xT_t[cc][:, kc * 128:(kc + 1) * 128],
                        wv_t[cc][:], start=(cc == 0), stop=(cc == 7))
                v3 = v_t[kc].rearrange("p (h e) -> p h e", h=NH)
                nc.vector.memset(v3[:, :, 64:128], 0.0)
                nc.vector.memset(v3[:, :, 64:65], 1.0)
                nc.vector.tensor_copy(
                    v3[:, :, 0:64],
                    ps.rearrange("p (h e) -> p h e", e=64))

            def proj_unit(pool, m2, j):
                js = slice(j * 512, (j + 1) * 512)
                py = pool.tile([128, 512], F32, tag="ps", name="py")
                for cl in range(4):
                    nc.tensor.matmul(
                        py[:], wp_t[cl][:, m2 * 128:(m2 + 1) * 128],
                        ao_t[cl][:, js], start=(cl == 0), stop=(cl == 3))
                yt = ytpool.tile([128, 512], F32, tag="yt", name="yt")
                if m2 % 2 == 0:
                    nc.vector.tensor_copy(yt[:], py[:])
                else:
                    nc.scalar.activation(yt[:], py[:], Copy)
                nc.sync.dma_start(out=yT[m2 * 128:(m2 + 1) * 128, js], in_=yt[:])

            # ---------------- prologue ----------------
            with tc.tile_pool(name="psAB", bufs=6, space="PSUM") as psAB:
                for jj in range(4):
                    qk_unit(psAB, 4, jj)
                qk_unit(psAB, 0, 0)
                for kc in range(8):
                    v_unit(psAB, kc)

            # ---------------- attention + interleaved fillers ----------------
            # per key-chunk kc: head-even exp on ACT (exact), head-odd exp on
            # DVE (Schraudolph). Separate engines overlap, and each parity's
            # ss WAR chain resolves at a steady per-engine pace so the
            # row-pair score MMs stay synchronized.
            with (
                tc.tile_pool(name="ssE", bufs=2, space="PSUM") as ssEp,
                tc.tile_pool(name="ssO", bufs=2, space="PSUM") as ssOp,
                tc.tile_pool(name="avE", bufs=1, space="PSUM") as avEp,
                tc.tile_pool(name="avO", bufs=1, space="PSUM") as avOp,
                tc.tile_pool(name="aux", bufs=2, space="PSUM") as aux,
            ):
                fill_q = {
                    0: ([("q", 1, 0)] + [("k", 5, jj) for jj in range(4)]
                        + [("q", 2, 0)] + [("k", 6, jj) for jj in range(4)]
                        + [("q", 3, 0)] + [("k", 7, jj) for jj in range(4)]
                        + [("q", m, 1) for m in range(4)]),
                    1: ([("proj", m2, 0) for m2 in range(8)]
                        + [("q", m, 2) for m in range(4)]),
                    2: ([("proj", m2, 1) for m2 in range(8)]
                        + [("q", m, 3) for m in range(4)]),
                    3: [("proj", m2, 2) for m2 in range(8)],
                }

                pend = [None]
                for j in range(4):
                    js = slice(j * 512, (j + 1) * 512)
                    queue = fill_q[j]
                    fstate = {"i": 0}

                    def filler():
                        if fstate["i"] < len(queue):
                            kind, a, b = queue[fstate["i"]]
                            fstate["i"] += 1
                            if kind == "proj":
                                proj_unit(aux, a, b)
                            elif kind == "v":
                                v_unit(aux, a)
                            else:  # "q" or "k" unit
                                qk_unit(aux, a, b, act_on_dve=(b % 2 == 1))

                    def sc_pair(p, kc):
                        ssE = ssEp.tile([128, 512], F32, tag="ssE", name="ssE")
                        ssO = ssOp.tile([128, 512], F32, tag="ssO", name="ssO")
                        ks = slice(kc * 128, (kc + 1) * 128)
                        nc.tensor.matmul(
                            ssE[:], qk_t[4 + p][0:64, ks],
                            qk_t[p][0:64, js], start=True, stop=True,
                            tile_position=(0, 0))
                        nc.tensor.matmul(
                            ssO[:], qk_t[4 + p][64:128, ks],
                            qk_t[p][64:128, js], start=True, stop=True,
                            tile_position=(64, 0))
                        return ssE, ssO

                    def exp_pair(ssE, ssO, kc):
                        ptE = ptpool.tile([128, 512], BF16, tag="ptE", name="ptE")
                        ptO = ptpool.tile([128, 512], BF16, tag="ptO", name="ptO")
                        nc.scalar.activation(ptE[:], ssE[:], Exp)
                        if kc < 15:
                            nc.vector.tensor_scalar(
                                ptO.bitcast(I16)[:], ssO[:], A_EXP, B_EXP, Mult, Add)
                        else:
                            # free the DVE for the cross-pair normalization
                            nc.scalar.activation(ptO[:], ssO[:], Exp)
                        return ptE, ptO

                    def av_pair(avE, avO, p, kc, pts):
                        ptE, ptO = pts[kc]
                        he, ho = 2 * p, 2 * p + 1
                        nc.tensor.matmul(
                            avE[:], v_t[kc][:, he * 128:(he + 1) * 128],
                            ptE[:], start=(kc == 0), stop=(kc == 15))
                        nc.tensor.matmul(
                            avO[:], v_t[kc][:, ho * 128:(ho + 1) * 128],
                            ptO[:], start=(kc == 0), stop=(kc == 15))

                    def norm_head(av, p, po, js_):
                        # av rows 0:64 = out^T, row 64 = Z; write ao rows po:po+64
                        z1 = rpool.tile([1, 512], F32, tag="z1", name="z1")
                        nc.vector.tensor_copy(z1[:], av[64:65, :])
                        r1 = rpool.tile([1, 512], F32, tag="r1", name="r1")
                        nc.vector.reciprocal_approx_fast(out=r1[:], in_=z1[:])
                        rb = rpool.tile([64, 512], F32, tag="rb", name="rb")
                        nc.gpsimd.partition_broadcast(rb[:], r1[:])
                        nc.vector.tensor_mul(
                            ao_t[p][po:po + 64, js_], av[0:64, :], rb[:])

                    for p in range(4):
                        pts = {}
                        avE = avO = None
                        for kc in range(16):
                            pts[kc] = exp_pair(*sc_pair(p, kc), kc)
                            if pend[0] is not None and kc in (0, 1):
                                pavE, pavO, pp, pjs, ppts = pend[0]
                                av_pair(pavE, pavO, pp, 14 + kc, ppts)
                                if kc == 1:
                                    norm_head(pavE, pp, 0, pjs)
                                    norm_head(pavO, pp, 64, pjs)
                                    pend[0] = None
                            if kc == 2:
                                avE = avEp.tile([128, 512], F32, tag="avE",
                                                name="avE")
                                avO = avOp.tile([128, 512], F32, tag="avO",
                                                name="avO")
                            if kc >= 2:
                                av_pair(avE, avO, p, kc - 2, pts)
                            if j == 0 and p == 0:
                                if kc < 8:
                                    v_unit(aux, kc + 8)
                                else:
                                    filler()
                            elif kc % 4 == 3:
                                filler()
                        pend[0] = (avE, avO, p, js, pts)

                    while fstate["i"] < len(queue):
                        filler()

                    if j == 3:
                        pavE, pavO, pp, pjs, ppts = pend[0]
                        av_pair(pavE, pavO, pp, 14, ppts)
                        av_pair(pavE, pavO, pp, 15, ppts)
                        norm_head(pavE, pp, 0, pjs)
                        norm_head(pavO, pp, 64, pjs)
                        pend[0] = None

                # tail: proj for j=3
                for m2 in range(8):
                    proj_unit(aux, m2, 3)

    nc.compile()
    return nc


def make_in_maps(x, w_qkv, b_qkv, w_proj):
    np_bf = mybir.dt.np(BF16)
    x = np.asarray(x, np.float32)
    w_qkv = np.asarray(w_qkv, np.float32)
    b_qkv = np.asarray(b_qkv, np.float32)
    w_proj = np.asarray(w_proj, np.float32)
    in_maps = []
    for c in range(NCORES):
        b, g = divmod(c, 2)
        h0 = g * NH
        qs = slice(h0 * D, h0 * D + CL)
        ks = slice(C + h0 * D, C + h0 * D + CL)
        vs = slice(2 * C + h0 * D, 2 * C + h0 * D + CL)
        wqk_m = np.concatenate([w_qkv[:, qs], w_qkv[:, ks]], axis=1)
        bq = b_qkv[qs] * SCALE
        bk = b_qkv[ks]
        bqk_m = np.concatenate([bq, bk]).reshape(8, 128).T  # [128, 8] col-chunks
        in_maps.append({
            "xT": np.ascontiguousarray(x[b].T).astype(np_bf),
            "wqk": np.ascontiguousarray(wqk_m).astype(np_bf),
            "wv": np.ascontiguousarray(w_qkv[:, vs]).astype(np_bf),
            "wp": np.ascontiguousarray(w_proj[h0 * D:h0 * D + CL, :]).astype(np_bf),
            "bqk": np.ascontiguousarray(bqk_m, np.float32),
        })
    return in_maps


def run(x, w_qkv, b_qkv, w_proj, b_proj, mm_dt=MM_DT, **spmd_kwargs):
    if "nc" not in _CACHE:
        _CACHE["nc"] = build_nc()
    nc = _CACHE["nc"]
    in_maps = make_in_maps(x, w_qkv, b_qkv, w_proj)
    res = run_bass_kernel_spmd(nc, in_maps, core_ids=list(range(NCORES)),
                               **spmd_kwargs)
    # v-bias passes through softmax averaging exactly (weights sum to 1),
    # so its projected contribution folds into the output bias on the host.
    b_eff = (np.asarray(b_proj, np.float64)
             + np.asarray(b_qkv, np.float64)[2 * C:] @ np.asarray(w_proj, np.float64)
             ).astype(np.float32)
    out = np.empty((B, N, C), np.float32)
    for b in range(B):
        acc = res.results[2 * b]["yT"] + res.results[2 * b + 1]["yT"]
        out[b] = acc.T + b_eff[None, :]
    return out, res


def kernel(x, w_qkv, b_qkv, w_proj, b_proj):
    out, _ = run(x, w_qkv, b_qkv, w_proj, b_proj)
    return out
